# revision 6
# baseline (speedup 1.0000x reference)
"""Causal self-attention (B=2, T=2048, C=1024, H=16, D=64) on 8 trn2 cores.

Sharding: core c handles batch b = c//4 and head group hg = c%4 (heads
4*hg .. 4*hg+3, as 2 pairs of 2 heads).  Each core computes q/k/v
projections for its 4 heads, causal-softmax attention, and a partial
output projection y_partial = O_heads @ Wo[:, heads].T (bf16).  The host
sums the 4 partials per batch in fp32 and adds the bias.

Dtypes / engine assignment (PE is the bottleneck engine):
  - x and all weights are cast to bf16 on the host (1 cycle/row matmuls,
    half the DMA traffic of fp32).
  - S = K^T Q runs in fp8e4m3 with MatmulPerfMode.DoubleRow: the D=64
    contraction is split into two 32-partition halves packed along the
    free dim ([32, 2, N] operands), so S costs 0.5 cycles/row instead of
    1 -- and uses the full 128-deep PE array instead of half.  Host-side
    channel permutation of Wq/Wk makes the PSUM->fp8 cast copies plain
    2-slice partition regroups.  Softmax renormalisation absorbs most of
    the fp8 noise; measured end-to-end rel err ~7e-3 (gate 2e-2).
  - V is projected directly in [T, D] row layout (lhsT = x^T tiles), so
    no PE transposes are needed for the PV matmul's vaug operand.
  - exp on ACT (PSUM fp32 -> SBUF bf16, scale=1/sqrt(C)); causal masking
    of the diagonal 128-strip on Pool (SBUF bf16); PSUM evacuations are
    DVE/ACT only (Pool and DMA cannot access PSUM).
  - O^T accumulates with a ones-augmented vaug so row 64 of the [65,512]
    PSUM tile is the softmax denominator (no separate reduction).

Emission interleaves a generic PE "filler" queue (y-projection chunks and
next-chunk projection pieces) into the ACT-bound attention inner loop so
the PE stream never starves behind exp.
"""
import numpy as np
import ml_dtypes

import concourse.tile as tile
import concourse.mybir as mybir
from concourse import bacc
from concourse.bass_utils import run_bass_kernel_spmd

FP = mybir.dt.float32
BF = mybir.dt.bfloat16
F8 = mybir.dt.float8e4
DR = mybir.MatmulPerfMode.DoubleRow
B, T, C = 2, 2048, 1024
H, D = 16, 64
SCALE = 1.0 / 32.0  # 1/sqrt(C)
N_CORES = 8
NKT = C // 128  # 8 K-tiles over the embedding dim
NTK = T // 128  # 16 Tk tiles
NI = T // 512  # 4 Tq chunks
EXP = mybir.ActivationFunctionType.Exp
NPBF = ml_dtypes.bfloat16

_nc_cache = {}


def build_kernel(repeats=1):
    key = repeats
    if key in _nc_cache:
        return _nc_cache[key]

    nc = bacc.Bacc("TRN2", target_bir_lowering=False, debug=False)

    xT_d = nc.dram_tensor("xT", [C, T], BF, kind="ExternalInput").ap()
    wqT_d = nc.dram_tensor("wqT", [C, 256], BF, kind="ExternalInput").ap()
    wkT_d = nc.dram_tensor("wkT", [C, 256], BF, kind="ExternalInput").ap()
    wvT_d = nc.dram_tensor("wvT", [C, 256], BF, kind="ExternalInput").ap()
    woT_d = nc.dram_tensor("woT", [256, C], BF, kind="ExternalInput").ap()
    y_d = nc.dram_tensor("y", [T, C], BF, kind="ExternalOutput").ap()

    # mask_big[p, y] = 1 iff y >= p + 384 : the diagonal 128-strip of tile j
    # (z = j*128 - I*512) is masked with slice [:, 384:512]
    mask_np = (
        np.arange(896)[None, :] >= (np.arange(128)[:, None] + 384)
    ).astype(NPBF)
    mask_d = nc.inline_tensor(mask_np, "mask_big").ap()
    ones_d = nc.inline_tensor(np.ones((128, 1), dtype=NPBF), "ones").ap()

    with tile.TileContext(nc) as tc:
        with (
            tc.tile_pool(name="persist", bufs=1) as pp,
            tc.tile_pool(name="xpool", bufs=16) as xpool,
            tc.tile_pool(name="ppool", bufs=4) as ppool,
            tc.tile_pool(name="spool", bufs=4) as spool,
            tc.tile_pool(name="ypool", bufs=4) as ypool,
            tc.tile_pool(name="ps_s", bufs=2, space="PSUM") as ps_s,
            tc.tile_pool(name="ps_o", bufs=2, space="PSUM") as ps_o,
            tc.tile_pool(name="ps_y", bufs=2, space="PSUM") as ps_y,
        ):
            # ---- critical-path DMAs first ----
            wq_big = pp.tile([128, NKT, 256], BF, tag="wq")
            nc.sync.dma_start(
                wq_big[:, :, :], wqT_d.rearrange("(n p) d -> p n d", p=128)
            )
            wq = [wq_big[:, kk, :] for kk in range(NKT)]
            xts_by_chunk = {0: [None] * NKT, 1: [None] * NKT}
            for kk in range(NKT):
                xt = xpool.tile([128, 512], BF, tag="xt", name=f"xtc0_{kk}")
                nc.sync.dma_start(xt[:], xT_d[kk * 128 : (kk + 1) * 128, 0:512])
                xts_by_chunk[0][kk] = xt
            wk_big = pp.tile([128, NKT, 256], BF, tag="wk")
            nc.sync.dma_start(
                wk_big[:, :, :], wkT_d.rearrange("(n p) d -> p n d", p=128)
            )
            wk = [wk_big[:, kk, :] for kk in range(NKT)]
            wv_big = pp.tile([128, NKT, 256], BF, tag="wv")
            nc.sync.dma_start(
                wv_big[:, :, :], wvT_d.rearrange("(n p) d -> p n d", p=128)
            )
            wv = [wv_big[:, kk, :] for kk in range(NKT)]
            for kk in range(NKT):
                xt = xpool.tile([128, 512], BF, tag="xt", name=f"xtc1_{kk}")
                nc.sync.dma_start(xt[:], xT_d[kk * 128 : (kk + 1) * 128, 512:1024])
                xts_by_chunk[1][kk] = xt
            wo_big = pp.tile([128, 2, C], BF, tag="wo")
            nc.sync.dma_start(
                wo_big[:, :, :], woT_d.rearrange("(n p) d -> p n d", p=128)
            )
            wo = [wo_big[:, kk, :] for kk in range(2)]

            mask = pp.tile([128, 896], BF, tag="mask")
            nc.sync.dma_start(mask[:], mask_d[:])
            ones_sb = pp.tile([128, 1], BF, tag="ones")
            nc.sync.dma_start(ones_sb[:], ones_d[:])

            # ---- persistent activations ----
            # qf8/kf8: fp8 DoubleRow layout [64, 2, 512] per (pair, chunk);
            # partitions 0:32 head-even, 32:64 head-odd; slot = D half.
            qf8 = [
                [pp.tile([64, 2, 512], F8, tag=f"q8{p}_{i}", name=f"q8{p}_{i}")
                 for i in range(NI)]
                for p in range(2)
            ]
            kf8 = [
                [pp.tile([64, 2, 512], F8, tag=f"k8{p}_{i}", name=f"k8{p}_{i}")
                 for i in range(NI)]
                for p in range(2)
            ]
            otstc = [
                [pp.tile([128, 512], BF, tag=f"ot{p}_{i}", name=f"otst{p}_{i}")
                 for i in range(NI)]
                for p in range(2)
            ]
            vaug = [
                [
                    pp.tile([128, 130], BF, tag=f"va{p}_{t}", name=f"vaug{p}_{t}")
                    for t in range(NTK)
                ]
                for p in range(2)
            ]
            # ones columns are persistent: init once on Pool (prologue)
            for p in range(2):
                for t in range(NTK):
                    nc.gpsimd.tensor_copy(vaug[p][t][:, 64:65], ones_sb[:])
                    nc.gpsimd.tensor_copy(vaug[p][t][:, 129:130], ones_sb[:])

            # ---- emission helpers ----
            R = [0]
            proj_q = []  # next-chunk projection pieces: MUST drain before
            #              the attention that reads them
            yproj_q = []  # y-projection chunks: anytime after their otstc

            def emit_xt_chunk(c):
                tiles = []
                for kk in range(NKT):
                    xt = xpool.tile([128, 512], BF, tag="xt",
                                    name=f"xtc{c}_{kk}_r{R[0]}")
                    nc.sync.dma_start(
                        xt[:], xT_d[kk * 128 : (kk + 1) * 128, c * 512 : c * 512 + 512]
                    )
                    tiles.append(xt)
                return tiles

            def emit_qk_piece(c, xts, wts, dst, nm, pair, **kw):
                ps = ps_y.tile([128, 512], FP, tag="ps_proj",
                               name=f"pspr{nm}{c}_{pair}_r{R[0]}")
                for kk in range(NKT):
                    nc.tensor.matmul(
                        ps[:],
                        lhsT=wts[kk][:, pair * 128 : pair * 128 + 128],
                        rhs=xts[kk][:],
                        start=(kk == 0),
                        stop=(kk == NKT - 1),
                    )
                for s in range(2):
                    nc.vector.tensor_copy(
                        dst[pair][c][:, s, :], ps[64 * s : 64 * s + 64, :]
                    )

            def emit_v_piece(c, xts, t, **kw):
                ps = ps_y.tile([128, 256], FP, tag="ps_proj",
                               name=f"psv{t}_r{R[0]}")
                for kk in range(NKT):
                    nc.tensor.matmul(
                        ps[:],
                        lhsT=xts[kk][:, (t % 4) * 128 : (t % 4) * 128 + 128],
                        rhs=wv[kk][:],
                        start=(kk == 0),
                        stop=(kk == NKT - 1),
                    )
                for pair in range(2):
                    for h in range(2):
                        nc.vector.tensor_copy(
                            vaug[pair][t][:, 65 * h : 65 * h + 64],
                            ps[:, pair * 128 + h * 64 : pair * 128 + h * 64 + 64],
                        )

            def proj_pieces(c, xts):
                ops = []
                for wts, dst, nm in ((wq, qf8, "q"), (wk, kf8, "k")):
                    for pair in range(2):
                        ops.append(
                            lambda c=c, xts=xts, wts=wts, dst=dst, nm=nm,
                            pair=pair, **kw: emit_qk_piece(c, xts, wts, dst, nm,
                                                           pair, **kw)
                        )
                for t in range(4 * c, 4 * c + 4):
                    ops.append(lambda c=c, xts=xts, t=t, **kw:
                               emit_v_piece(c, xts, t, **kw))
                return ops

            def emit_yproj_chunk(t, nch, on_act=False):
                ps = ps_y.tile([128, 512], FP, tag="ps_proj",
                               name=f"psy{t}_{nch}_r{R[0]}")
                for pair in range(2):
                    nc.tensor.matmul(
                        ps[:],
                        lhsT=otstc[pair][t // 4][
                            :, (t % 4) * 128 : (t % 4) * 128 + 128
                        ],
                        rhs=wo[pair][:, nch * 512 : nch * 512 + 512],
                        start=(pair == 0),
                        stop=(pair == 1),
                    )
                yt = ypool.tile([128, 512], BF, tag="yout", name=f"yt{t}_{nch}_r{R[0]}")
                if on_act or nch == 0:
                    nc.scalar.copy(yt[:], ps[:])
                else:
                    nc.vector.tensor_copy(yt[:], ps[:])
                nc.sync.dma_start(
                    y_d[t * 128 : (t + 1) * 128, nch * 512 : nch * 512 + 512],
                    yt[:],
                )

            def maybe_fill():
                if proj_q:
                    proj_q.pop(0)()
                elif yproj_q:
                    yproj_q.pop(0)()

            def emit_attention(I):
                jmax = 4 * I + 4
                for pair in range(2):
                    oT = [None, None]
                    for h in (1, 0):
                        oT[h] = ps_o.tile([65, 512], FP, tag="oT",
                                          name=f"o{I}_{pair}_{h}_r{R[0]}")
                        hsl = slice(32 * h, 32 * h + 32)
                        for jb in range(jmax // 2):
                            j0 = 2 * jb
                            diag = j0 >= 4 * I  # both tiles in diagonal region
                            zs = [max(0, (j0 + dj) * 128 - I * 512) for dj in range(2)]
                            s_ps = ps_s.tile([128, 1024], FP, tag="s",
                                             name=f"s{I}_{pair}_{h}_{jb}_r{R[0]}")
                            for dj in range(2):
                                j = j0 + dj
                                z = zs[dj]
                                nc.tensor.matmul(
                                    s_ps[:, dj * 512 + z : dj * 512 + 512],
                                    lhsT=kf8[pair][j // 4][
                                        hsl, :, (j % 4) * 128 : (j % 4) * 128 + 128
                                    ],
                                    rhs=qf8[pair][I][hsl, :, z:512],
                                    start=True,
                                    stop=True,
                                    perf_mode=DR,
                                )
                            p_sb = ppool.tile([128, 1024], BF, tag="p",
                                              name=f"p{I}_{pair}_{h}_{jb}_r{R[0]}")
                            if not diag:
                                nc.scalar.activation(p_sb[:], s_ps[:], EXP,
                                                     scale=SCALE)
                            else:
                                # trimmed: columns below the causal boundary
                                # were never computed
                                for dj in range(2):
                                    lo = dj * 512 + zs[dj]
                                    hi = dj * 512 + 512
                                    nc.scalar.activation(
                                        p_sb[:, lo:hi], s_ps[:, lo:hi], EXP,
                                        scale=SCALE,
                                    )
                            for dj in range(2):
                                j = j0 + dj
                                z = zs[dj]
                                if j >= 4 * I:
                                    # triangular strip at the causal boundary
                                    ssl2 = slice(dj * 512 + z, dj * 512 + z + 128)
                                    nc.gpsimd.tensor_mul(
                                        p_sb[:, ssl2], p_sb[:, ssl2],
                                        mask[:, 384:512],
                                    )
                                nc.tensor.matmul(
                                    oT[h][:, z:512],
                                    lhsT=vaug[pair][j][:, 65 * h : 65 * h + 65],
                                    rhs=p_sb[:, dj * 512 + z : dj * 512 + 512],
                                    start=(j == 0),
                                    stop=(j == jmax - 1),
                                )
                            maybe_fill()
                    # normalize: O^T[0:64] * (1/rowsum) into the stacked chunk
                    for h in (1, 0):
                        recip = spool.tile([1, 512], FP, tag="recip",
                                           name=f"rc{I}_{pair}_{h}_r{R[0]}")
                        nc.vector.reciprocal(recip[:], oT[h][64:65, :])
                        bcast = spool.tile([64, 512], FP, tag="bcast",
                                           name=f"bc{I}_{pair}_{h}_r{R[0]}")
                        nc.gpsimd.partition_broadcast(bcast[:], recip[:])
                        if h == 0:
                            nc.vector.tensor_mul(
                                otstc[pair][I][0:64, :], oT[h][0:64, :], bcast[:]
                            )
                        else:
                            onrm = spool.tile([64, 512], BF, tag="onrm",
                                              name=f"on{I}_{pair}_r{R[0]}")
                            nc.vector.tensor_mul(onrm[:], oT[h][0:64, :], bcast[:])
                            # partition shift 0->64 needs a DMA
                            nc.sync.dma_start(otstc[pair][I][64:128, :], onrm[:])
                for t in range(4 * I, 4 * I + 4):
                    for nch in range(2):
                        yproj_q.append(
                            lambda t=t, nch=nch, **kw: emit_yproj_chunk(t, nch, **kw)
                        )

            # ---- interleaved emission: proj chunks 0,1 inline, then
            # attention(1..3) with proj(c+1) pieces and yproj chunks popped
            # as PE fillers inside the (ACT-bound) attention inner loop.
            # attention(0) (shortest) runs last.
            def run_rep(rep):
                R[0] = rep
                if rep == 0:
                    xts0, xts1 = xts_by_chunk[0], xts_by_chunk[1]
                else:
                    xts0 = emit_xt_chunk(0)
                    xts1 = emit_xt_chunk(1)
                for op in proj_pieces(0, xts0):
                    op()
                for op in proj_pieces(1, xts1):
                    op()
                for c in range(1, NI):
                    if c + 1 < NI:
                        xts_n = emit_xt_chunk(c + 1)
                        proj_q.extend(proj_pieces(c + 1, xts_n))
                    emit_attention(c)
                    while proj_q:  # attention(c+1) needs proj(c+1) complete
                        proj_q.pop(0)()
                emit_attention(0)
                while yproj_q:
                    yproj_q.pop(0)(on_act=True)  # tail: ACT is idle here

            for rep in range(repeats):
                run_rep(rep)

    nc.compile()
    _nc_cache[key] = nc
    return nc


def _perm256():
    """Channel order so proj PSUM partitions are [s*64 + h*32 + d32]:
    partitions 0:32 h0 dhalf0, 32:64 h1 dhalf0, 64:96 h0 dhalf1,
    96:128 h1 dhalf1 per 128-row pair block."""
    perm = np.zeros(256, dtype=np.int64)
    for p in range(2):
        for s in range(2):
            for h in range(2):
                for d in range(32):
                    perm[p * 128 + s * 64 + h * 32 + d] = (
                        p * 128 + h * 64 + s * 32 + d
                    )
    return perm


def make_in_maps(x, Wq, Wk, Wv, Wo):
    x = np.asarray(x, dtype=np.float32)
    Wq = np.asarray(Wq, dtype=np.float32)
    Wk = np.asarray(Wk, dtype=np.float32)
    Wv = np.asarray(Wv, dtype=np.float32)
    Wo = np.asarray(Wo, dtype=np.float32)
    perm = _perm256()
    in_maps = []
    for c in range(N_CORES):
        b, hg = c // 4, c % 4
        sl = slice(256 * hg, 256 * hg + 256)
        in_maps.append(
            {
                "xT": np.ascontiguousarray(x[b].T.astype(NPBF)),
                "wqT": np.ascontiguousarray(Wq[sl, :][perm].T.astype(NPBF)),
                "wkT": np.ascontiguousarray(Wk[sl, :][perm].T.astype(NPBF)),
                "wvT": np.ascontiguousarray(Wv[sl, :].T.astype(NPBF)),
                "woT": np.ascontiguousarray(Wo[:, sl].T.astype(NPBF)),
            }
        )
    return in_maps


def run_spmd(in_maps, trace=False, repeats=1, **kw):
    nc = build_kernel(repeats)
    return run_bass_kernel_spmd(nc, in_maps, list(range(N_CORES)), trace=trace, **kw)


def gather(results, bo):
    bo = np.asarray(bo, dtype=np.float32)
    y = np.empty((B, T, C), dtype=np.float32)
    for b in range(B):
        acc = results[4 * b]["y"].astype(np.float32)
        for g in range(1, 4):
            acc = acc + results[4 * b + g]["y"].astype(np.float32)
        y[b] = acc + bo[None, :]
    return y


def kernel(x, Wq, Wk, Wv, Wo, bo):
    res = run_spmd(make_in_maps(x, Wq, Wk, Wv, Wo))
    return gather(res.results, bo)


# revision 50
# speedup vs baseline: 1.6928x; 1.6928x over previous
"""Causal self-attention (B=2, T=2048, C=1024, H=16, D=64) on 8 trn2 cores.

Sharding: core c handles batch b = c//4 and head group hg = c%4 (heads
4*hg .. 4*hg+3, as 2 pairs of 2 heads).  Each core computes q/k/v
projections for its 4 heads, causal-softmax attention, and a partial
output projection y_partial = O_heads @ Wo[:, heads].T (bf16).  The host
sums the 4 partials per batch in fp32 and adds the bias.

Dtypes / engine assignment (PE is the bottleneck engine):
  - x and all weights are cast to bf16 on the host (1 cycle/row matmuls,
    half the DMA traffic of fp32).
  - S = K^T Q runs in fp8e4m3 with MatmulPerfMode.DoubleRow: the D=64
    contraction is split into two 32-partition halves packed along the
    free dim ([32, 2, N] operands), so S costs 0.5 cycles/row instead of
    1 -- and uses the full 128-deep PE array instead of half.  Host-side
    channel permutation of Wq/Wk makes the PSUM->fp8 cast copies plain
    2-slice partition regroups.  Softmax renormalisation absorbs most of
    the fp8 noise; measured end-to-end rel err ~7e-3 (gate 2e-2).
  - V is projected directly in [T, D] row layout (lhsT = x^T tiles), so
    no PE transposes are needed for the PV matmul's vaug operand.
  - exp on ACT (PSUM fp32 -> SBUF bf16, scale=1/sqrt(C)); causal masking
    of the diagonal 128-strip on Pool (SBUF bf16); PSUM evacuations are
    DVE/ACT only (Pool and DMA cannot access PSUM).
  - O^T accumulates with a ones-augmented vaug so row 64 of the [65,512]
    PSUM tile is the softmax denominator (no separate reduction).

Emission interleaves a generic PE "filler" queue (y-projection chunks and
next-chunk projection pieces) into the ACT-bound attention inner loop so
the PE stream never starves behind exp.
"""
import numpy as np
import ml_dtypes

import concourse.tile as tile
import concourse.mybir as mybir
from concourse import bacc
from concourse.bass_utils import run_bass_kernel_spmd

FP = mybir.dt.float32
BF = mybir.dt.bfloat16
F8 = mybir.dt.float8e4
DR = mybir.MatmulPerfMode.DoubleRow
B, T, C = 2, 2048, 1024
H, D = 16, 64
SCALE = 1.0 / 32.0  # 1/sqrt(C)
N_CORES = 8
NKT = C // 128  # 8 K-tiles over the embedding dim
NTK = T // 128  # 16 Tk tiles
NI = T // 512  # 4 Tq chunks
EXP = mybir.ActivationFunctionType.Exp
NPBF = ml_dtypes.bfloat16
NPF8 = ml_dtypes.float8_e4m3

_nc_cache = {}


def build_kernel(repeats=1):
    key = repeats
    if key in _nc_cache:
        return _nc_cache[key]

    nc = bacc.Bacc("TRN2", target_bir_lowering=False, debug=False)

    xT_d = nc.dram_tensor("xT", [C, T], BF, kind="ExternalInput").ap()
    xT8_d = nc.dram_tensor("xT8", [C, T], F8, kind="ExternalInput").ap()
    wqT_d = nc.dram_tensor("wqT", [C, 256], F8, kind="ExternalInput").ap()
    wkT_d = nc.dram_tensor("wkT", [C, 256], F8, kind="ExternalInput").ap()
    wvT_d = nc.dram_tensor("wvT", [C, 256], BF, kind="ExternalInput").ap()
    woT_d = nc.dram_tensor("woT", [256, C], BF, kind="ExternalInput").ap()
    y_d = nc.dram_tensor("y", [T, C], BF, kind="ExternalOutput").ap()

    # mask_big[p, y] = 1 iff y >= p + 384 : the diagonal 128-strip of tile j
    # (z = j*128 - I*512) is masked with slice [:, 384:512]
    mask_np = (
        np.arange(896)[None, :] >= (np.arange(128)[:, None] + 384)
    ).astype(NPBF)
    mask_d = nc.inline_tensor(mask_np, "mask_big").ap()
    ones_d = nc.inline_tensor(np.ones((128, 1), dtype=NPBF), "ones").ap()

    with tile.TileContext(nc) as tc:
        with (
            tc.tile_pool(name="persist", bufs=1) as pp,
            tc.tile_pool(name="xpool", bufs=3) as xpool,
            tc.tile_pool(name="ppool", bufs=8) as ppool,
            tc.tile_pool(name="spool", bufs=8) as spool,
            tc.tile_pool(name="ypool", bufs=8) as ypool,
            tc.tile_pool(name="ps_s", bufs=2, space="PSUM") as ps_s,
            tc.tile_pool(name="ps_o", bufs=2, space="PSUM") as ps_o,
            tc.tile_pool(name="ps_y", bufs=2, space="PSUM") as ps_y,
        ):
            # ---- critical-path DMAs first ----
            # wq/wk fp8 in DoubleRow pairing: [128, m, two, 256] with slot
            # `two` = K-tile 2m+two of the C=1024 contraction
            wq_big = pp.tile([128, 4, 2, 256], F8, tag="wq")
            nc.sync.dma_start(
                wq_big[:, :, :, :],
                wqT_d.rearrange("(m two p) d -> p m two d", p=128, two=2),
            )
            def load_x8_chunk(c, rep=""):
                x8big = xpool.tile([128, 4, 2, 512], F8, tag="x8",
                                   name=f"x8c{c}{rep}")
                nc.sync.dma_start(
                    x8big[:, :, :, :],
                    xT8_d[:, c * 512 : c * 512 + 512].rearrange(
                        "(m two p) t -> p m two t", p=128, two=2
                    ),
                )
                return [x8big[:, m, :, :] for m in range(4)]

            def load_xt_chunk(c, rep=""):
                xbig = xpool.tile([128, NKT, 512], BF, tag="xt",
                                  name=f"xtc{c}{rep}")
                nc.sync.dma_start(
                    xbig[:, :, :],
                    xT_d[:, c * 512 : c * 512 + 512].rearrange(
                        "(n p) t -> p n t", p=128
                    ),
                )
                return [xbig[:, kk, :] for kk in range(NKT)]

            x8s_by_chunk = {0: load_x8_chunk(0)}
            wk_big = pp.tile([128, 4, 2, 256], F8, tag="wk")
            nc.sync.dma_start(
                wk_big[:, :, :, :],
                wkT_d.rearrange("(m two p) d -> p m two d", p=128, two=2),
            )
            wv_big = pp.tile([128, NKT, 256], BF, tag="wv")
            nc.sync.dma_start(
                wv_big[:, :, :], wvT_d.rearrange("(n p) d -> p n d", p=128)
            )
            wv = [wv_big[:, kk, :] for kk in range(NKT)]
            xts_by_chunk = {0: load_xt_chunk(0)}
            x8s_by_chunk[1] = load_x8_chunk(1)
            xts_by_chunk[1] = load_xt_chunk(1)
            wo_big = pp.tile([128, 2, C], BF, tag="wo")
            nc.sync.dma_start(
                wo_big[:, :, :], woT_d.rearrange("(n p) d -> p n d", p=128)
            )
            wo = [wo_big[:, kk, :] for kk in range(2)]

            mask = pp.tile([128, 896], BF, tag="mask")
            nc.sync.dma_start(mask[:], mask_d[:])
            ones_sb = pp.tile([128, 1], BF, tag="ones")
            nc.sync.dma_start(ones_sb[:], ones_d[:])

            # ---- persistent activations ----
            # qf8/kf8: fp8 DoubleRow layout [64, 2, 512] per (pair, chunk);
            # partitions 0:32 head-even, 32:64 head-odd; slot = D half.
            qf8 = [
                [pp.tile([64, 2, 512], F8, tag=f"q8{p}_{i}", name=f"q8{p}_{i}")
                 for i in range(NI)]
                for p in range(2)
            ]
            kf8 = [
                [pp.tile([64, 2, 512], F8, tag=f"k8{p}_{i}", name=f"k8{p}_{i}")
                 for i in range(NI)]
                for p in range(2)
            ]
            otstc = [
                [pp.tile([128, 512], BF, tag=f"ot{p}_{i}", name=f"otst{p}_{i}")
                 for i in range(NI)]
                for p in range(2)
            ]
            # vaug [128, 2, 65]: slot h = [V_h | ones]; one strided DVE copy
            # fills both heads' V from the [T,256] projection PSUM.
            vaug = [
                [
                    pp.tile([128, 2, 65], BF, tag=f"va{p}_{t}", name=f"vaug{p}_{t}")
                    for t in range(NTK)
                ]
                for p in range(2)
            ]
            # ones columns are persistent: init once on Pool (prologue)
            for p in range(2):
                for t in range(NTK):
                    for s in range(2):
                        nc.gpsimd.tensor_copy(vaug[p][t][:, s, 64:65], ones_sb[:])

            # ---- emission helpers ----
            R = [0]
            # PE filler queues by priority: v pieces of the CURRENT chunk
            # (needed by this attention's diagonal), q/k pieces of the NEXT
            # chunk (needed before the next attention), y-projection chunks
            # (anytime; carry across rep boundaries).
            vq, qkq, yq = [], [], []

            def emit_xt_chunk(c):
                r = f"_r{R[0]}"
                return load_xt_chunk(c, r), load_x8_chunk(c, r)

            def emit_qk_piece(c, x8s, wbig, dst, nm, pair, **kw):
                ps = ps_y.tile([128, 512], FP, tag="ps_proj",
                               name=f"pspr{nm}{c}_{pair}_r{R[0]}")
                for m in range(4):
                    nc.tensor.matmul(
                        ps[:],
                        lhsT=wbig[:, m, :, pair * 128 : pair * 128 + 128],
                        rhs=x8s[m][:, :, :],
                        start=(m == 0),
                        stop=(m == 3),
                        perf_mode=DR,
                    )
                    if m == 1:
                        yield
                for s in range(2):
                    nc.vector.tensor_copy(
                        dst[pair][c][:, s, :], ps[64 * s : 64 * s + 64, :]
                    )

            def emit_v_piece(c, xts, t, **kw):
                ps = ps_y.tile([128, 2, 2, 64], FP, tag="ps_proj",
                               name=f"psv{t}_r{R[0]}")
                for kk in range(NKT):
                    nc.tensor.matmul(
                        ps[:, :, :, :],
                        lhsT=xts[kk][:, (t % 4) * 128 : (t % 4) * 128 + 128],
                        rhs=wv[kk][:],
                        start=(kk == 0),
                        stop=(kk == NKT - 1),
                    )
                    if kk % 3 == 2:
                        yield
                for pair in range(2):
                    nc.vector.tensor_copy(
                        vaug[pair][t][:, :, 0:64], ps[:, pair, :, :]
                    )

            def qk_pieces(c, x8s):
                return [
                    lambda c=c, x8s=x8s, wbig=wbig, dst=dst, nm=nm, pair=pair,
                    **kw: emit_qk_piece(c, x8s, wbig, dst, nm, pair, **kw)
                    for wbig, dst, nm in ((wq_big, qf8, "q"), (wk_big, kf8, "k"))
                    for pair in range(2)
                ]

            def v_pieces(c, xts):
                return [
                    lambda c=c, xts=xts, t=t, **kw: emit_v_piece(c, xts, t, **kw)
                    for t in range(4 * c, 4 * c + 4)
                ]

            # filler machinery: factories produce generators that yield
            # between matmuls, so attention pops sub-microsecond PE quanta
            cur = []  # in-flight generator

            def _start_next(**kw):
                for q in (vq, qkq, yq):
                    if q:
                        cur[:] = [q.pop(0)(**kw)]
                        return True
                return False

            def maybe_fill(n=1):
                done = 0
                while done < n:
                    if not cur and not _start_next():
                        return
                    try:
                        next(cur[0])
                        done += 1
                    except StopIteration:
                        cur[:] = []

            def drain_queue(q, **kw):
                # finish the in-flight generator, then every piece in q
                if cur:
                    for _ in cur[0]:
                        pass
                    cur[:] = []
                while q:
                    for _ in q.pop(0)(**kw):
                        pass

            def drain_all(**kw):
                drain_queue(vq, **kw)
                drain_queue(qkq, **kw)
                drain_queue(yq, **kw)

            ybig = {}  # t -> [128, 1024] staging tile; DMA'd once both
            # halves are cast (halves the y DMA count)

            def emit_yproj_chunk(t, nch, on_act=False):
                ps = ps_y.tile([128, 512], FP, tag="ps_proj",
                               name=f"psy{t}_{nch}_r{R[0]}")
                for pair in range(2):
                    nc.tensor.matmul(
                        ps[:],
                        lhsT=otstc[pair][t // 4][
                            :, (t % 4) * 128 : (t % 4) * 128 + 128
                        ],
                        rhs=wo[pair][:, nch * 512 : nch * 512 + 512],
                        start=(pair == 0),
                        stop=(pair == 1),
                    )
                    if pair == 0:
                        yield
                if nch == 0:
                    ybig[t] = ypool.tile([128, 1024], BF, tag="yout",
                                         name=f"yt{t}_r{R[0]}")
                yt = ybig[t]
                if on_act:
                    nc.scalar.copy(yt[:, nch * 512 : nch * 512 + 512], ps[:])
                else:
                    nc.vector.tensor_copy(yt[:, nch * 512 : nch * 512 + 512], ps[:])
                if nch == 1:
                    nc.sync.dma_start(
                        y_d[t * 128 : (t + 1) * 128, :], yt[:]
                    )

            def emit_attention(I):
                jmax = 4 * I + 4
                did_vguard = [False]

                def nfill():
                    return 2 if (vq or qkq) else 1

                def emit_pv(oT_h, h, pair, jb, p_sb, segs):
                    for dj in range(2):
                        j = 2 * jb + dj
                        z, lo = segs[dj]
                        nc.tensor.matmul(
                            oT_h[:, z:512],
                            lhsT=vaug[pair][j][:, h, :],
                            rhs=p_sb[:, lo : lo + 512 - z],
                            start=(j == 0),
                            stop=(j == jmax - 1),
                        )

                for pair in range(2):
                    # the two heads of the pair run in LOCKSTEP: two
                    # independent S->exp->PV chains interleaved, so ACT
                    # always has a second exp stream while PE works
                    oT = [
                        ps_o.tile([65, 512], FP, tag="oT",
                                  name=f"o{I}_{pair}_{h}_r{R[0]}")
                        for h in range(2)
                    ]
                    pend = [None, None]
                    for jb in range(jmax // 2):
                        j0 = 2 * jb
                        diag = j0 >= 4 * I  # both tiles in diagonal region
                        if diag and not did_vguard[0]:
                            # this chunk's vaug tiles are read by the diag
                            # PV: force any remaining v pieces out now
                            did_vguard[0] = True
                            drain_queue(vq)
                        # segments: (z = causal trim, lo = column base in
                        # s_ps/p_sb).  Diag jb packs dj1 directly after dj0's
                        # valid range so ONE exp covers both tiles.
                        if not diag:
                            segs = [(0, 0), (0, 512)]
                        else:
                            z0 = j0 * 128 - I * 512
                            segs = [(z0, z0), (z0 + 128, 512)]
                        p_sbs = [None, None]
                        for h in range(2):
                            hsl = slice(32 * h, 32 * h + 32)
                            s_ps = ps_s.tile([128, 1024], FP, tag="s",
                                             name=f"s{I}_{pair}_{h}_{jb}_r{R[0]}")
                            for dj in range(2):
                                j = j0 + dj
                                z, lo = segs[dj]
                                nc.tensor.matmul(
                                    s_ps[:, lo : lo + 512 - z],
                                    lhsT=kf8[pair][j // 4][
                                        hsl, :, (j % 4) * 128 : (j % 4) * 128 + 128
                                    ],
                                    rhs=qf8[pair][I][hsl, :, z:512],
                                    start=True,
                                    stop=True,
                                    perf_mode=DR,
                                )
                            p_sb = ppool.tile([128, 1024], BF, tag="p",
                                              name=f"p{I}_{pair}_{h}_{jb}_r{R[0]}")
                            p_sbs[h] = p_sb
                            if not diag:
                                nc.scalar.activation(p_sb[:], s_ps[:], EXP,
                                                     scale=SCALE)
                            else:
                                lo0 = segs[0][1]
                                hi1 = segs[1][1] + 512 - segs[1][0]
                                nc.scalar.activation(
                                    p_sb[:, lo0:hi1], s_ps[:, lo0:hi1], EXP,
                                    scale=SCALE,
                                )
                            if diag:
                                for dj in range(2):
                                    z, lo = segs[dj]
                                    nc.gpsimd.tensor_mul(
                                        p_sb[:, lo : lo + 128],
                                        p_sb[:, lo : lo + 128],
                                        mask[:, 384:512],
                                    )
                        for h in range(2):
                            maybe_fill(nfill())
                            if pend[h] is not None:
                                emit_pv(oT[h], h, pair, *pend[h])
                            pend[h] = (jb, p_sbs[h], segs)
                    for h in range(2):
                        maybe_fill(nfill())
                        emit_pv(oT[h], h, pair, *pend[h])
                        # normalize this head now: O^T[0:64] * (1/rowsum);
                        # frees the oT PSUM buffer as early as possible
                        recip = spool.tile([1, 512], FP, tag="recip",
                                           name=f"rc{I}_{pair}_{h}_r{R[0]}")
                        nc.vector.reciprocal(recip[:], oT[h][64:65, :])
                        bcast = spool.tile([64, 512], FP, tag="bcast",
                                           name=f"bc{I}_{pair}_{h}_r{R[0]}")
                        nc.gpsimd.partition_broadcast(bcast[:], recip[:])
                        # DVE writes at a shifted partition base, so h=1
                        # lands directly in otstc rows 64:128
                        nc.vector.tensor_mul(
                            otstc[pair][I][64 * h : 64 * h + 64, :],
                            oT[h][0:64, :], bcast[:],
                        )
                for t in range(4 * I, 4 * I + 4):
                    for nch in range(2):
                        yq.append(
                            lambda t=t, nch=nch, **kw: emit_yproj_chunk(t, nch, **kw)
                        )

            # ---- interleaved emission: proj chunks 0,1 inline, then
            # attention(1..3) with proj(c+1) pieces and yproj chunks popped
            # as PE fillers inside the (ACT-bound) attention inner loop.
            # attention(0) (shortest) runs last.
            def run_rep(rep):
                R[0] = rep
                if rep == 0:
                    xts0, x8s0 = xts_by_chunk[0], x8s_by_chunk[0]
                    xts1, x8s1 = xts_by_chunk[1], x8s_by_chunk[1]
                else:
                    xts0, x8s0 = emit_xt_chunk(0)
                    xts1, x8s1 = emit_xt_chunk(1)
                for op in qk_pieces(0, x8s0) + v_pieces(0, xts0):
                    for _ in op():
                        pass
                qkq.extend(qk_pieces(1, x8s1))
                xts_c = {1: xts1}
                for c in range(NI):
                    if c >= 1:
                        # q/k of chunk c must be complete before attention(c)
                        drain_queue(qkq)
                        vq.extend(v_pieces(c, xts_c[c]))
                        if c + 1 < NI:
                            xts_n, x8s_n = emit_xt_chunk(c + 1)
                            xts_c[c + 1] = xts_n
                            qkq.extend(qk_pieces(c + 1, x8s_n))
                    emit_attention(c)
                # leftover y-projection chunks carry into the next rep's
                # attention windows; the final rep drains them on ACT

            for rep in range(repeats):
                run_rep(rep)
            drain_all(on_act=True)  # tail: ACT is idle here

    nc.compile()
    _nc_cache[key] = nc
    return nc


def _perm256():
    """Channel order so proj PSUM partitions are [s*64 + h*32 + d32]:
    partitions 0:32 h0 dhalf0, 32:64 h1 dhalf0, 64:96 h0 dhalf1,
    96:128 h1 dhalf1 per 128-row pair block."""
    perm = np.zeros(256, dtype=np.int64)
    for p in range(2):
        for s in range(2):
            for h in range(2):
                for d in range(32):
                    perm[p * 128 + s * 64 + h * 32 + d] = (
                        p * 128 + h * 64 + s * 32 + d
                    )
    return perm


def make_in_maps(x, Wq, Wk, Wv, Wo):
    x = np.asarray(x, dtype=np.float32)
    Wq = np.asarray(Wq, dtype=np.float32)
    Wk = np.asarray(Wk, dtype=np.float32)
    Wv = np.asarray(Wv, dtype=np.float32)
    Wo = np.asarray(Wo, dtype=np.float32)
    perm = _perm256()
    in_maps = []
    for c in range(N_CORES):
        b, hg = c // 4, c % 4
        sl = slice(256 * hg, 256 * hg + 256)
        xT = x[b].T
        in_maps.append(
            {
                "xT": np.ascontiguousarray(xT.astype(NPBF)),
                "xT8": np.ascontiguousarray(xT.astype(NPF8)),
                "wqT": np.ascontiguousarray(Wq[sl, :][perm].T.astype(NPF8)),
                "wkT": np.ascontiguousarray(Wk[sl, :][perm].T.astype(NPF8)),
                "wvT": np.ascontiguousarray(Wv[sl, :].T.astype(NPBF)),
                "woT": np.ascontiguousarray(Wo[:, sl].T.astype(NPBF)),
            }
        )
    return in_maps


def run_spmd(in_maps, trace=False, repeats=1, **kw):
    nc = build_kernel(repeats)
    return run_bass_kernel_spmd(nc, in_maps, list(range(N_CORES)), trace=trace, **kw)


def gather(results, bo):
    bo = np.asarray(bo, dtype=np.float32)
    y = np.empty((B, T, C), dtype=np.float32)
    for b in range(B):
        acc = results[4 * b]["y"].astype(np.float32)
        for g in range(1, 4):
            acc = acc + results[4 * b + g]["y"].astype(np.float32)
        y[b] = acc + bo[None, :]
    return y


def kernel(x, Wq, Wk, Wv, Wo, bo):
    res = run_spmd(make_in_maps(x, Wq, Wk, Wv, Wo))
    return gather(res.results, bo)


# revision 51
# speedup vs baseline: 1.6970x; 1.0025x over previous
"""Causal self-attention (B=2, T=2048, C=1024, H=16, D=64) on 8 trn2 cores.

Sharding: core c handles batch b = c//4 and head group hg = c%4 (heads
4*hg .. 4*hg+3, as 2 pairs of 2 heads).  Each core computes q/k/v
projections for its 4 heads, causal-softmax attention, and a partial
output projection y_partial = O_heads @ Wo[:, heads].T (bf16).  The host
sums the 4 partials per batch in fp32 and adds the bias.

Dtypes / engine assignment (PE is the bottleneck engine):
  - x and all weights are cast to bf16 on the host (1 cycle/row matmuls,
    half the DMA traffic of fp32).
  - S = K^T Q and the q/k projections run in fp8e4m3 with
    MatmulPerfMode.DoubleRow ([K, 2, M] operands, 0.5 cycles/row): the
    D=64 contraction of S is split into two 32-partition halves, so it
    uses the full 128-deep PE array; the q/k projections pack adjacent
    C-K-tiles into the two planes.  Host-side channel permutation of
    Wq/Wk makes the PSUM->fp8 cast copies plain 2-slice partition
    regroups.  Softmax renormalisation absorbs most of the fp8 noise;
    measured end-to-end rel err 1.35e-2 (gate 2e-2; all-bf16 fallback
    measured 3.4e-3 at ~30% more PE time).
  - V is projected directly in [T, D] row layout (lhsT = x^T tiles), so
    no PE transposes are needed for the PV matmul's vaug operand.
  - exp on ACT (PSUM fp32 -> SBUF bf16, scale=1/sqrt(C)); causal masking
    of the diagonal 128-strip on Pool (SBUF bf16); PSUM evacuations are
    DVE/ACT only (Pool and DMA cannot access PSUM).
  - O^T accumulates with a ones-augmented vaug so row 64 of the [65,512]
    PSUM tile is the softmax denominator (no separate reduction).

Emission interleaves a generic PE "filler" queue (y-projection chunks and
next-chunk projection pieces) into the ACT-bound attention inner loop so
the PE stream never starves behind exp.
"""
import numpy as np
import ml_dtypes

import concourse.tile as tile
import concourse.mybir as mybir
from concourse import bacc
from concourse.bass_utils import run_bass_kernel_spmd

FP = mybir.dt.float32
BF = mybir.dt.bfloat16
F8 = mybir.dt.float8e4
DR = mybir.MatmulPerfMode.DoubleRow
B, T, C = 2, 2048, 1024
H, D = 16, 64
SCALE = 1.0 / 32.0  # 1/sqrt(C)
N_CORES = 8
NKT = C // 128  # 8 K-tiles over the embedding dim
NTK = T // 128  # 16 Tk tiles
NI = T // 512  # 4 Tq chunks
EXP = mybir.ActivationFunctionType.Exp
NPBF = ml_dtypes.bfloat16
NPF8 = ml_dtypes.float8_e4m3

_nc_cache = {}


def build_kernel(repeats=1):
    key = repeats
    if key in _nc_cache:
        return _nc_cache[key]

    nc = bacc.Bacc("TRN2", target_bir_lowering=False, debug=False)

    xT_d = nc.dram_tensor("xT", [C, T], BF, kind="ExternalInput").ap()
    xT8_d = nc.dram_tensor("xT8", [C, T], F8, kind="ExternalInput").ap()
    wqT_d = nc.dram_tensor("wqT", [C, 256], F8, kind="ExternalInput").ap()
    wkT_d = nc.dram_tensor("wkT", [C, 256], F8, kind="ExternalInput").ap()
    wvT_d = nc.dram_tensor("wvT", [C, 256], BF, kind="ExternalInput").ap()
    woT_d = nc.dram_tensor("woT", [256, C], BF, kind="ExternalInput").ap()
    y_d = nc.dram_tensor("y", [T, C], BF, kind="ExternalOutput").ap()

    # mask_big[p, y] = 1 iff y >= p + 384 : the diagonal 128-strip of tile j
    # (z = j*128 - I*512) is masked with slice [:, 384:512]
    mask_np = (
        np.arange(896)[None, :] >= (np.arange(128)[:, None] + 384)
    ).astype(NPBF)
    mask_d = nc.inline_tensor(mask_np, "mask_big").ap()
    ones_d = nc.inline_tensor(np.ones((128, 1), dtype=NPBF), "ones").ap()

    with tile.TileContext(nc) as tc:
        with (
            tc.tile_pool(name="persist", bufs=1) as pp,
            tc.tile_pool(name="xpool", bufs=3) as xpool,
            tc.tile_pool(name="ppool", bufs=8) as ppool,
            tc.tile_pool(name="spool", bufs=8) as spool,
            tc.tile_pool(name="ypool", bufs=8) as ypool,
            tc.tile_pool(name="ps_s", bufs=2, space="PSUM") as ps_s,
            tc.tile_pool(name="ps_o", bufs=2, space="PSUM") as ps_o,
            tc.tile_pool(name="ps_y", bufs=2, space="PSUM") as ps_y,
        ):
            # ---- critical-path DMAs first ----
            # wq/wk fp8 in DoubleRow pairing: [128, m, two, 256] with slot
            # `two` = K-tile 2m+two of the C=1024 contraction
            wq_big = pp.tile([128, 4, 2, 256], F8, tag="wq")
            nc.sync.dma_start(
                wq_big[:, :, :, :],
                wqT_d.rearrange("(m two p) d -> p m two d", p=128, two=2),
            )
            def load_x8_chunk(c, rep=""):
                x8big = xpool.tile([128, 4, 2, 512], F8, tag="x8",
                                   name=f"x8c{c}{rep}")
                nc.sync.dma_start(
                    x8big[:, :, :, :],
                    xT8_d[:, c * 512 : c * 512 + 512].rearrange(
                        "(m two p) t -> p m two t", p=128, two=2
                    ),
                )
                return [x8big[:, m, :, :] for m in range(4)]

            def load_xt_chunk(c, rep=""):
                xbig = xpool.tile([128, NKT, 512], BF, tag="xt",
                                  name=f"xtc{c}{rep}")
                nc.sync.dma_start(
                    xbig[:, :, :],
                    xT_d[:, c * 512 : c * 512 + 512].rearrange(
                        "(n p) t -> p n t", p=128
                    ),
                )
                return [xbig[:, kk, :] for kk in range(NKT)]

            x8s_by_chunk = {0: load_x8_chunk(0)}
            wk_big = pp.tile([128, 4, 2, 256], F8, tag="wk")
            nc.sync.dma_start(
                wk_big[:, :, :, :],
                wkT_d.rearrange("(m two p) d -> p m two d", p=128, two=2),
            )
            wv_big = pp.tile([128, NKT, 256], BF, tag="wv")
            nc.sync.dma_start(
                wv_big[:, :, :], wvT_d.rearrange("(n p) d -> p n d", p=128)
            )
            wv = [wv_big[:, kk, :] for kk in range(NKT)]
            xts_by_chunk = {0: load_xt_chunk(0)}
            x8s_by_chunk[1] = load_x8_chunk(1)
            xts_by_chunk[1] = load_xt_chunk(1)
            wo_big = pp.tile([128, 2, C], BF, tag="wo")
            nc.sync.dma_start(
                wo_big[:, :, :], woT_d.rearrange("(n p) d -> p n d", p=128)
            )
            wo = [wo_big[:, kk, :] for kk in range(2)]

            mask = pp.tile([128, 896], BF, tag="mask")
            nc.sync.dma_start(mask[:], mask_d[:])
            ones_sb = pp.tile([128, 1], BF, tag="ones")
            nc.sync.dma_start(ones_sb[:], ones_d[:])

            # ---- persistent activations ----
            # qf8/kf8: fp8 DoubleRow layout [64, 2, 512] per (pair, chunk);
            # partitions 0:32 head-even, 32:64 head-odd; slot = D half.
            qf8 = [
                [pp.tile([64, 2, 512], F8, tag=f"q8{p}_{i}", name=f"q8{p}_{i}")
                 for i in range(NI)]
                for p in range(2)
            ]
            kf8 = [
                [pp.tile([64, 2, 512], F8, tag=f"k8{p}_{i}", name=f"k8{p}_{i}")
                 for i in range(NI)]
                for p in range(2)
            ]
            otstc = [
                [pp.tile([128, 512], BF, tag=f"ot{p}_{i}", name=f"otst{p}_{i}")
                 for i in range(NI)]
                for p in range(2)
            ]
            # vaug [128, 2, 65]: slot h = [V_h | ones]; one strided DVE copy
            # fills both heads' V from the [T,256] projection PSUM.
            vaug = [
                [
                    pp.tile([128, 2, 65], BF, tag=f"va{p}_{t}", name=f"vaug{p}_{t}")
                    for t in range(NTK)
                ]
                for p in range(2)
            ]
            # ones columns are persistent: init once on Pool (prologue)
            for p in range(2):
                for t in range(NTK):
                    for s in range(2):
                        nc.gpsimd.tensor_copy(vaug[p][t][:, s, 64:65], ones_sb[:])

            # ---- emission helpers ----
            R = [0]
            # PE filler queues by priority: v pieces of the CURRENT chunk
            # (needed by this attention's diagonal), q/k pieces of the NEXT
            # chunk (needed before the next attention), y-projection chunks
            # (anytime; carry across rep boundaries).
            vq, qkq, yq = [], [], []

            def emit_xt_chunk(c):
                r = f"_r{R[0]}"
                return load_xt_chunk(c, r), load_x8_chunk(c, r)

            def emit_qk_piece(c, x8s, wbig, dst, nm, pair, **kw):
                ps = ps_y.tile([128, 512], FP, tag="ps_proj",
                               name=f"pspr{nm}{c}_{pair}_r{R[0]}")
                for m in range(4):
                    nc.tensor.matmul(
                        ps[:],
                        lhsT=wbig[:, m, :, pair * 128 : pair * 128 + 128],
                        rhs=x8s[m][:, :, :],
                        start=(m == 0),
                        stop=(m == 3),
                        perf_mode=DR,
                    )
                    if m == 1:
                        yield
                for s in range(2):
                    nc.vector.tensor_copy(
                        dst[pair][c][:, s, :], ps[64 * s : 64 * s + 64, :]
                    )

            def emit_v_piece(c, xts, t, **kw):
                ps = ps_y.tile([128, 2, 2, 64], FP, tag="ps_proj",
                               name=f"psv{t}_r{R[0]}")
                for kk in range(NKT):
                    nc.tensor.matmul(
                        ps[:, :, :, :],
                        lhsT=xts[kk][:, (t % 4) * 128 : (t % 4) * 128 + 128],
                        rhs=wv[kk][:],
                        start=(kk == 0),
                        stop=(kk == NKT - 1),
                    )
                    if kk % 3 == 2:
                        yield
                for pair in range(2):
                    nc.vector.tensor_copy(
                        vaug[pair][t][:, :, 0:64], ps[:, pair, :, :]
                    )

            def qk_pieces(c, x8s):
                return [
                    lambda c=c, x8s=x8s, wbig=wbig, dst=dst, nm=nm, pair=pair,
                    **kw: emit_qk_piece(c, x8s, wbig, dst, nm, pair, **kw)
                    for wbig, dst, nm in ((wq_big, qf8, "q"), (wk_big, kf8, "k"))
                    for pair in range(2)
                ]

            def v_pieces(c, xts):
                return [
                    lambda c=c, xts=xts, t=t, **kw: emit_v_piece(c, xts, t, **kw)
                    for t in range(4 * c, 4 * c + 4)
                ]

            # filler machinery: factories produce generators that yield
            # between matmuls, so attention pops sub-microsecond PE quanta
            cur = []  # in-flight generator

            def _start_next(**kw):
                for q in (vq, qkq, yq):
                    if q:
                        cur[:] = [q.pop(0)(**kw)]
                        return True
                return False

            def maybe_fill(n=1):
                done = 0
                while done < n:
                    if not cur and not _start_next():
                        return
                    try:
                        next(cur[0])
                        done += 1
                    except StopIteration:
                        cur[:] = []

            def drain_queue(q, **kw):
                # finish the in-flight generator, then every piece in q
                if cur:
                    for _ in cur[0]:
                        pass
                    cur[:] = []
                while q:
                    for _ in q.pop(0)(**kw):
                        pass

            def drain_all(**kw):
                drain_queue(vq, **kw)
                drain_queue(qkq, **kw)
                drain_queue(yq, **kw)

            ybig = {}  # t -> [128, 1024] staging tile; DMA'd once both
            # halves are cast (halves the y DMA count)

            def emit_yproj_chunk(t, nch, on_act=False):
                ps = ps_y.tile([128, 512], FP, tag="ps_proj",
                               name=f"psy{t}_{nch}_r{R[0]}")
                for pair in range(2):
                    nc.tensor.matmul(
                        ps[:],
                        lhsT=otstc[pair][t // 4][
                            :, (t % 4) * 128 : (t % 4) * 128 + 128
                        ],
                        rhs=wo[pair][:, nch * 512 : nch * 512 + 512],
                        start=(pair == 0),
                        stop=(pair == 1),
                    )
                    if pair == 0:
                        yield
                if nch == 0:
                    ybig[t] = ypool.tile([128, 1024], BF, tag="yout",
                                         name=f"yt{t}_r{R[0]}")
                yt = ybig[t]
                if on_act:
                    nc.scalar.copy(yt[:, nch * 512 : nch * 512 + 512], ps[:])
                else:
                    nc.vector.tensor_copy(yt[:, nch * 512 : nch * 512 + 512], ps[:])
                if nch == 1:
                    nc.sync.dma_start(
                        y_d[t * 128 : (t + 1) * 128, :], yt[:]
                    )

            def emit_attention(I):
                jmax = 4 * I + 4
                did_vguard = [False]

                def nfill():
                    return 2 if (vq or qkq) else 1

                def emit_pv(oT_h, h, pair, jb, p_sb, segs):
                    for dj in range(2):
                        j = 2 * jb + dj
                        z, lo = segs[dj]
                        nc.tensor.matmul(
                            oT_h[:, z:512],
                            lhsT=vaug[pair][j][:, h, :],
                            rhs=p_sb[:, lo : lo + 512 - z],
                            start=(j == 0),
                            stop=(j == jmax - 1),
                        )

                for pair in range(2):
                    # the two heads of the pair run in LOCKSTEP: two
                    # independent S->exp->PV chains interleaved, so ACT
                    # always has a second exp stream while PE works
                    oT = [
                        ps_o.tile([65, 512], FP, tag="oT",
                                  name=f"o{I}_{pair}_{h}_r{R[0]}")
                        for h in range(2)
                    ]
                    pend = [None, None]
                    for jb in range(jmax // 2):
                        j0 = 2 * jb
                        diag = j0 >= 4 * I  # both tiles in diagonal region
                        if diag and not did_vguard[0]:
                            # this chunk's vaug tiles are read by the diag
                            # PV: force any remaining v pieces out now
                            did_vguard[0] = True
                            drain_queue(vq)
                        # segments: (z = causal trim, lo = column base in
                        # s_ps/p_sb).  Diag jb packs dj1 directly after dj0's
                        # valid range so ONE exp covers both tiles.
                        if not diag:
                            segs = [(0, 0), (0, 512)]
                        else:
                            z0 = j0 * 128 - I * 512
                            segs = [(z0, z0), (z0 + 128, 512)]
                        p_sbs = [None, None]
                        for h in range(2):
                            hsl = slice(32 * h, 32 * h + 32)
                            s_ps = ps_s.tile([128, 1024], FP, tag="s",
                                             name=f"s{I}_{pair}_{h}_{jb}_r{R[0]}")
                            for dj in range(2):
                                j = j0 + dj
                                z, lo = segs[dj]
                                nc.tensor.matmul(
                                    s_ps[:, lo : lo + 512 - z],
                                    lhsT=kf8[pair][j // 4][
                                        hsl, :, (j % 4) * 128 : (j % 4) * 128 + 128
                                    ],
                                    rhs=qf8[pair][I][hsl, :, z:512],
                                    start=True,
                                    stop=True,
                                    perf_mode=DR,
                                )
                            p_sb = ppool.tile([128, 1024], BF, tag="p",
                                              name=f"p{I}_{pair}_{h}_{jb}_r{R[0]}")
                            p_sbs[h] = p_sb
                            if not diag:
                                nc.scalar.activation(p_sb[:], s_ps[:], EXP,
                                                     scale=SCALE)
                            else:
                                lo0 = segs[0][1]
                                hi1 = segs[1][1] + 512 - segs[1][0]
                                nc.scalar.activation(
                                    p_sb[:, lo0:hi1], s_ps[:, lo0:hi1], EXP,
                                    scale=SCALE,
                                )
                            if diag:
                                for dj in range(2):
                                    z, lo = segs[dj]
                                    nc.gpsimd.tensor_mul(
                                        p_sb[:, lo : lo + 128],
                                        p_sb[:, lo : lo + 128],
                                        mask[:, 384:512],
                                    )
                        for h in range(2):
                            maybe_fill(nfill())
                            if pend[h] is not None:
                                emit_pv(oT[h], h, pair, *pend[h])
                            pend[h] = (jb, p_sbs[h], segs)
                    for h in range(2):
                        maybe_fill(nfill())
                        emit_pv(oT[h], h, pair, *pend[h])
                        # normalize this head now: O^T[0:64] * (1/rowsum);
                        # frees the oT PSUM buffer as early as possible
                        recip = spool.tile([1, 512], FP, tag="recip",
                                           name=f"rc{I}_{pair}_{h}_r{R[0]}")
                        nc.vector.reciprocal(recip[:], oT[h][64:65, :])
                        bcast = spool.tile([64, 512], FP, tag="bcast",
                                           name=f"bc{I}_{pair}_{h}_r{R[0]}")
                        nc.gpsimd.partition_broadcast(bcast[:], recip[:])
                        # DVE writes at a shifted partition base, so h=1
                        # lands directly in otstc rows 64:128
                        nc.vector.tensor_mul(
                            otstc[pair][I][64 * h : 64 * h + 64, :],
                            oT[h][0:64, :], bcast[:],
                        )
                for t in range(4 * I, 4 * I + 4):
                    for nch in range(2):
                        yq.append(
                            lambda t=t, nch=nch, **kw: emit_yproj_chunk(t, nch, **kw)
                        )

            # ---- interleaved emission: proj chunks 0,1 inline, then
            # attention(1..3) with proj(c+1) pieces and yproj chunks popped
            # as PE fillers inside the (ACT-bound) attention inner loop.
            # attention(0) (shortest) runs last.
            def run_rep(rep):
                R[0] = rep
                if rep == 0:
                    xts0, x8s0 = xts_by_chunk[0], x8s_by_chunk[0]
                    xts1, x8s1 = xts_by_chunk[1], x8s_by_chunk[1]
                else:
                    xts0, x8s0 = emit_xt_chunk(0)
                    xts1, x8s1 = emit_xt_chunk(1)
                for op in qk_pieces(0, x8s0) + v_pieces(0, xts0):
                    for _ in op():
                        pass
                qkq.extend(qk_pieces(1, x8s1))
                xts_c = {1: xts1}
                for c in range(NI):
                    if c >= 1:
                        # q/k of chunk c must be complete before attention(c)
                        drain_queue(qkq)
                        vq.extend(v_pieces(c, xts_c[c]))
                        if c + 1 < NI:
                            xts_n, x8s_n = emit_xt_chunk(c + 1)
                            xts_c[c + 1] = xts_n
                            qkq.extend(qk_pieces(c + 1, x8s_n))
                    emit_attention(c)
                # leftover y-projection chunks carry into the next rep's
                # attention windows; the final rep drains them on ACT

            for rep in range(repeats):
                run_rep(rep)
            drain_all(on_act=True)  # tail: ACT is idle here

    nc.compile()
    _nc_cache[key] = nc
    return nc


def _perm256():
    """Channel order so proj PSUM partitions are [s*64 + h*32 + d32]:
    partitions 0:32 h0 dhalf0, 32:64 h1 dhalf0, 64:96 h0 dhalf1,
    96:128 h1 dhalf1 per 128-row pair block."""
    perm = np.zeros(256, dtype=np.int64)
    for p in range(2):
        for s in range(2):
            for h in range(2):
                for d in range(32):
                    perm[p * 128 + s * 64 + h * 32 + d] = (
                        p * 128 + h * 64 + s * 32 + d
                    )
    return perm


def make_in_maps(x, Wq, Wk, Wv, Wo):
    x = np.asarray(x, dtype=np.float32)
    Wq = np.asarray(Wq, dtype=np.float32)
    Wk = np.asarray(Wk, dtype=np.float32)
    Wv = np.asarray(Wv, dtype=np.float32)
    Wo = np.asarray(Wo, dtype=np.float32)
    perm = _perm256()
    in_maps = []
    for c in range(N_CORES):
        b, hg = c // 4, c % 4
        sl = slice(256 * hg, 256 * hg + 256)
        xT = x[b].T
        in_maps.append(
            {
                "xT": np.ascontiguousarray(xT.astype(NPBF)),
                "xT8": np.ascontiguousarray(xT.astype(NPF8)),
                "wqT": np.ascontiguousarray(Wq[sl, :][perm].T.astype(NPF8)),
                "wkT": np.ascontiguousarray(Wk[sl, :][perm].T.astype(NPF8)),
                "wvT": np.ascontiguousarray(Wv[sl, :].T.astype(NPBF)),
                "woT": np.ascontiguousarray(Wo[:, sl].T.astype(NPBF)),
            }
        )
    return in_maps


def run_spmd(in_maps, trace=False, repeats=1, **kw):
    nc = build_kernel(repeats)
    return run_bass_kernel_spmd(nc, in_maps, list(range(N_CORES)), trace=trace, **kw)


def gather(results, bo):
    bo = np.asarray(bo, dtype=np.float32)
    y = np.empty((B, T, C), dtype=np.float32)
    for b in range(B):
        acc = results[4 * b]["y"].astype(np.float32)
        for g in range(1, 4):
            acc = acc + results[4 * b + g]["y"].astype(np.float32)
        y[b] = acc + bo[None, :]
    return y


def kernel(x, Wq, Wk, Wv, Wo, bo):
    res = run_spmd(make_in_maps(x, Wq, Wk, Wv, Wo))
    return gather(res.results, bo)
